# revision 1
# baseline (speedup 1.0000x reference)
"""Compressed-KV GPT-2 attention block on 8 TRN2 NeuronCores.

Sharding: batch x head-group. Core c: batch b = c//4, heads 4*(c%4)..4*(c%4)+4.

Key structural idea vs a naive port: the KV compressor is linear + low-rank
(C=32 < hd=64), so attention runs entirely in the compressed C-space and the
decompressors fold into host-side weights:
  scores = q k_dec^T/8 = (q wk_d^T/8) (k wk_c)^T  -> w_q' = w_q wk_d^T/8 [D,32]
                                                     w_k' = w_k wk_c     [D,32]
  out_h  = (P v_c) (wv_d w_proj_h)               -> w_proj' = wv_d_h w_proj_h [32,D]
This shrinks qkv from 6 to 4 m-blocks (q'|k_c are 32 rows/head), c_proj
contracts over C=32 (4 heads -> but packed as 2 head-pairs with dead rows),
and the v decompress disappears (only v_comp = v wv_c stays on device).

Device pipeline per core (bf16 matmuls -> fp32 PSUM), per seq-block sb:
  qkv^T   = w' chunks @ hidden^T        (m-blocks: q'0-3 | kc0-3 | v01 | v23)
  v_comp  = vT-ktile^T @ blockdiag(wv_c pair)  -> [kpos, 32|1|32|1] vco tiles
  S^T     = kc'^T-slices^T @ q'^T  (K=32, PE quadrant per head) -> exp -> E
            (band tiles: single exp + gpsimd diag-mask multiply)
  attn_c  = vco^T @ E   (K=128, M=33: row 32 = ones = softmax denom;
            head-odd lands at PSUM partitions 64-96 via tile_position)
  norm    = gpsimd copy -> DVE recip(den) -> DMA partition-bcast -> DVE mult
  out^T  += w_proj'-pair^T @ attn_pair  (2 accum matmuls, dead rows are 0)
"""

import sys

if "/opt/trn_rl_repo" not in sys.path:
    sys.path.insert(0, "/opt/trn_rl_repo")

import numpy as np
import ml_dtypes

BF16 = ml_dtypes.bfloat16

B, S, D = 2, 2048, 1024
H, hd, C = 16, 64, 32
NCORES = 8
HPC = 4            # heads per core
SB = 512           # free-dim block (PSUM bank / max moving cols)
NSB = S // SB      # 4 seq blocks of 512
NKT = S // 128     # 16 key tiles of 128
DC = D // 128      # 8 contraction chunks for qkv
VW = C + 1         # v_comp + ones column width per head slot

_cache = {}


def _build():
    import concourse.bacc as bacc
    import concourse.tile as tile
    import concourse.mybir as mybir

    dt = mybir.dt
    f32, bf16 = dt.float32, dt.bfloat16
    Exp = mybir.ActivationFunctionType.Exp
    Copy = mybir.ActivationFunctionType.Copy
    mult = mybir.AluOpType.mult

    nc = bacc.Bacc("TRN2", target_bir_lowering=False, debug=False, num_devices=NCORES)

    hidden_t = nc.dram_tensor("hidden_t", [D, S], bf16, kind="ExternalInput")
    w_qkv = nc.dram_tensor("w_qkv", [D, 4 * 128], bf16, kind="ExternalInput")
    b_qkv = nc.dram_tensor("b_qkv", [128, 4], f32, kind="ExternalInput")
    wvc2 = nc.dram_tensor("wvc2", [128, 128], bf16, kind="ExternalInput")
    wpj = nc.dram_tensor("wpj", [2, 128, D], bf16, kind="ExternalInput")
    maskd = nc.dram_tensor("maskd", [128, 128], bf16, kind="ExternalInput")
    out_t = nc.dram_tensor("out_t", [D, S], bf16, kind="ExternalOutput")

    with tile.TileContext(nc) as tc:
        with (
            tc.tile_pool(name="persist", bufs=1) as pp,
            tc.tile_pool(name="work", bufs=4) as wp,
            tc.tile_pool(name="epool", bufs=24) as ep,
            tc.tile_pool(name="npool", bufs=4) as npo,
            tc.tile_pool(name="ostage", bufs=3) as op,
            tc.tile_pool(name="dscr", bufs=4, space="DRAM") as dr,
            tc.tile_pool(name="ps_s", bufs=4, space="PSUM") as ps_s,
            tc.tile_pool(name="ps_o", bufs=2, space="PSUM") as ps_o,
            tc.tile_pool(name="ps_p", bufs=2, space="PSUM") as ps_p,
        ):
            # ---- weights first (small), hidden per-sb in consumption order ----
            bias = pp.tile([128, 4], f32, tag="bias", name="bias")
            nc.sync.dma_start(bias[:], b_qkv.ap())
            wq = []
            for d in range(DC):
                w = pp.tile([128, 4 * 128], bf16, tag=f"wq{d}", name=f"wq{d}")
                nc.sync.dma_start(w[:], w_qkv.ap()[d * 128:(d + 1) * 128, :])
                wq.append(w)
            hT = [pp.tile([128, S], bf16, tag=f"hT{d}", name=f"hT{d}") for d in range(DC)]
            sl0 = slice(0, SB)
            for d in range(DC):
                nc.sync.dma_start(hT[d][:, sl0], hidden_t.ap()[d * 128:(d + 1) * 128, sl0])

            wvc2_t = pp.tile([128, 128], bf16, tag="wvc2", name="wvc2")
            nc.sync.dma_start(wvc2_t[:], wvc2.ap())
            maskt = pp.tile([128, 128], bf16, tag="mask", name="maskt")
            nc.sync.dma_start(maskt[:], maskd.ap())
            wpj_t = []
            for p in range(2):
                t = pp.tile([128, D], bf16, tag=f"wpj{p}", name=f"wpj{p}")
                nc.sync.dma_start(t[:], wpj.ap()[p])
                wpj_t.append(t)

            # qkv m-block destinations
            qcT = pp.tile([128, S], bf16, tag="qcT", name="qcT")
            kcT = pp.tile([128, S], bf16, tag="kcT", name="kcT")
            vT = [pp.tile([128, S], bf16, tag=f"vT{p}", name=f"vT{p}") for p in range(2)]
            dests = [qcT, kcT, vT[0], vT[1]]

            # v_comp (+ones) per pair: slot kt: [v_even(32) | 1 | v_odd(32) | 1]
            vco = [pp.tile([128, NKT * 2 * VW], bf16, tag=f"vco{p}", name=f"vco{p}")
                   for p in range(2)]
            for p in range(2):
                nc.vector.memset(vco[p][:], 1.0)

            # attn_c packed per pair: rows 0-31 head even, 64-95 head odd,
            # rows 32-63 / 96-127 stay zero (dead rows for the c_proj matmul)
            attn = [pp.tile([128, S], bf16, tag=f"attn{p}", name=f"attn{p}")
                    for p in range(2)]
            for p in range(2):
                nc.vector.memset(attn[p][:], 0.0)

            # PE operands must start at SBUF partition 0/32/64 — head 3 lives
            # at 96, so keep a DMA-shifted copy of its q'/k_c rows at base 32
            qc3 = pp.tile([64, S], bf16, tag="qc3", name="qc3")
            kc3 = pp.tile([64, S], bf16, tag="kc3", name="kc3")

            def emit_cproj(sb):
                sl = slice(sb * SB, (sb + 1) * SB)
                for mb in range(DC):
                    psp = ps_p.tile([128, SB], f32, tag="psP", name="psP")
                    for p in range(2):
                        nc.tensor.matmul(
                            psp[:],
                            wpj_t[p][:, mb * 128:(mb + 1) * 128],
                            attn[p][:, sl],
                            start=(p == 0),
                            stop=(p == 1),
                        )
                    stage = op.tile([128, SB], bf16, tag="stage", name="stage")
                    if mb % 2 == 0:
                        nc.vector.tensor_copy(stage[:], psp[:])
                    else:
                        nc.scalar.activation(stage[:], psp[:], Copy)
                    nc.sync.dma_start(out_t.ap()[mb * 128:(mb + 1) * 128, sl], stage[:])

            # ==== phase 1: all qkv (long dense PE stream to hold max clock) ====
            for sb in range(NSB):
                sl = slice(sb * SB, (sb + 1) * SB)
                if sb > 0:
                    for d in range(DC):
                        nc.sync.dma_start(
                            hT[d][:, sl],
                            hidden_t.ap()[d * 128:(d + 1) * 128, sl],
                        )
                for mb in range(4):
                    ps = ps_s.tile([128, SB], f32, tag="psS", name="psS")
                    for d in range(DC):
                        nc.tensor.matmul(
                            ps[:],
                            wq[d][:, mb * 128:(mb + 1) * 128],
                            hT[d][:, sl],
                            start=(d == 0),
                            stop=(d == DC - 1),
                        )
                    nc.vector.tensor_scalar_add(
                        out=dests[mb][:, sl],
                        in0=ps[:],
                        scalar1=bias[:, mb:mb + 1],
                    )
                nc.sync.dma_start(qc3[32:64, sl], qcT[96:128, sl])
                nc.sync.dma_start(kc3[32:64, sl], kcT[96:128, sl])

            # ==== phase 2: all v_comp ====
            for p in range(2):
                for kt in range(NKT):
                    ps = ps_s.tile([128, SB], f32, tag="psS", name="psC")
                    nc.tensor.matmul(
                        ps[:, 0:64],
                        vT[p][:, kt * 128:(kt + 1) * 128],
                        wvc2_t[:, p * 64:(p + 1) * 64],
                    )
                    nc.vector.tensor_copy(
                        vco[p][:, kt * 2 * VW:(kt + 1) * 2 * VW]
                        .rearrange("p (two c) -> p two c", two=2)[:, :, 0:C],
                        ps[:, 0:64].rearrange("p (two c) -> p two c", two=2),
                    )

            # ==== phase 3: attention, software-pipelined one group deep ====
            # groups in sb-descending order: biggest exp batches first, the
            # small sb=0 groups + final c_proj form a short tail.
            def scores_emitters(sb, h, es):
                """One closure per scores tile; each appends its E to es."""
                nkb = 4 * sb + 4
                if h == 3:
                    kc_src, qc_src, hsl = kc3, qc3, slice(32, 64)
                else:
                    kc_src, qc_src, hsl = kcT, qcT, slice(32 * h, 32 * h + 32)

                def one(kb):
                    def emit():
                        r = kb - 4 * sb
                        c0 = max(r, 0) * 128
                        psc = ps_s.tile([128, SB], f32, tag="psS", name="psS")
                        nc.tensor.matmul(
                            psc[:, c0:SB],
                            kc_src[hsl, kb * 128:(kb + 1) * 128],
                            qc_src[hsl, sb * SB + c0:(sb + 1) * SB],
                        )
                        e = ep.tile([128, SB], bf16, tag="E", name="e")
                        nc.scalar.activation(e[:, c0:SB], psc[:, c0:SB], Exp)
                        if r >= 0:
                            nc.vector.tensor_tensor(
                                e[:, c0:c0 + 128], e[:, c0:c0 + 128],
                                maskt[:], mult
                            )
                        es.append((e, c0))
                    return emit
                return [one(kb) for kb in range(nkb)]

            pso_of = {}

            def pv_emitters(sb, h, es):
                nkb = 4 * sb + 4
                p, off = h // 2, 64 * (h % 2)
                if h % 2 == 0:
                    pso_of[(sb, p)] = ps_o.tile([128, SB], f32, tag="psO",
                                                name="psO")
                pso = pso_of[(sb, p)]

                def one(kb):
                    def emit():
                        e, c0 = es[kb]
                        nc.tensor.matmul(
                            pso[off:off + C + 1, c0:SB],
                            vco[p][:, kb * 2 * VW + (h % 2) * VW:
                                   kb * 2 * VW + (h % 2) * VW + VW],
                            e[:, c0:SB],
                            start=(kb == 0),
                            stop=(kb == nkb - 1),
                        )
                        if kb == nkb - 1 and h % 2 == 1:
                            emit_norm(sb, p, pso)
                    return emit
                return [one(kb) for kb in range(nkb)]

            def emit_norm(sb, p, pso):
                # bcast den via DRAM bounce ([128,4] shape for a cheap DVE
                # reciprocal), then num * (1/den); DMAs ride gpsimd SWDGE
                sl = slice(sb * SB, (sb + 1) * SB)
                nsb = npo.tile([128, SB], bf16, tag="nsb", name="nsb")
                nc.vector.tensor_copy(nsb[0:C + 1, :], pso[0:C + 1, :])
                nc.vector.tensor_copy(nsb[64:64 + C + 1, :],
                                      pso[64:64 + C + 1, :])
                recb = wp.tile([128, SB], bf16, tag="recb", name="recb")
                for o in (0, 64):
                    denc = wp.tile([128, 4], bf16, tag="denc", name="denc")
                    nc.gpsimd.dma_start(denc[:], nsb[o + C:o + C + 1, :])
                    with nc.allow_low_precision(reason="softmax denom recip"):
                        nc.vector.reciprocal(denc[:], denc[:])
                    recd = dr.tile([SB], bf16, tag="recd", name="recd")
                    nc.gpsimd.dma_start(
                        recd[:].rearrange("(p j) -> p j", p=128), denc[:]
                    )
                    nc.gpsimd.dma_start(
                        recb[o:o + C, :],
                        recd[:].unsqueeze(0).to_broadcast([C, SB]),
                    )
                nc.vector.tensor_tensor(
                    attn[p][0:C, sl], nsb[0:C, :], recb[0:C, :], mult
                )
                nc.vector.tensor_tensor(
                    attn[p][64:64 + C, sl], nsb[64:64 + C, :],
                    recb[64:64 + C, :], mult
                )

            groups = [(sb, h) for sb in range(NSB - 1, -1, -1) for h in range(HPC)]
            prev = None          # (sb, h, es) awaiting PV
            cproj_due = []       # sbs whose attn is complete, c_proj pending
            for sb, h in groups:
                es = []
                s_list = scores_emitters(sb, h, es)
                pv_list = pv_emitters(*prev) if prev is not None else []
                # 2 scores lead-in, then alternate 1:1 (matches the ~2 PE
                # matmuls per Act exp rate), remainder appended
                si = 0
                for lead in range(min(2, len(s_list))):
                    s_list[si](); si += 1
                pi = 0
                while si < len(s_list) or pi < len(pv_list):
                    if si < len(s_list):
                        s_list[si](); si += 1
                    if pi < len(pv_list):
                        pv_list[pi](); pi += 1
                if prev is not None and prev[1] == 3:
                    cproj_due.append(prev[0])
                # keep the last two c_projs in reserve: they are the only
                # dependency-free PE work left to hide the final normalize
                # chains of the small sb=0/1 groups
                if len(cproj_due) > 1:
                    emit_cproj(cproj_due.pop(0))
                prev = (sb, h, es)
            for emit in pv_emitters(*prev):
                emit()
            cproj_due.append(prev[0])
            for sb in cproj_due:
                emit_cproj(sb)

    nc.compile()
    return nc


def _prep_inputs(hidden_states, w_attn, b_attn, wk_c, wv_c, wk_d, wv_d, w_proj):
    """Per-core input maps: fold the low-rank KV compressors into weights.

      w_q' = w_q @ wk_d^T / sqrt(hd)   [D, C]   (scores contract over C)
      w_k' = w_k @ wk_c                [D, C]
      w_proj'_h = wv_d_h @ w_proj_h    [C, D]   (c_proj contracts over C)
    """
    f8 = np.float64
    hidden_T = [np.ascontiguousarray(hidden_states[b].T).astype(BF16) for b in range(B)]
    scale = 1.0 / np.sqrt(hd)
    wq_h = lambda h: (w_attn[:, h * hd:(h + 1) * hd].astype(f8)
                      @ wk_d[h].astype(f8).T * scale).astype(np.float32)
    bq_h = lambda h: (b_attn[h * hd:(h + 1) * hd].astype(f8)
                      @ wk_d[h].astype(f8).T * scale).astype(np.float32)
    wk_h = lambda h: (w_attn[:, D + h * hd:D + (h + 1) * hd].astype(f8)
                      @ wk_c[h].astype(f8)).astype(np.float32)
    bk_h = lambda h: (b_attn[D + h * hd:D + (h + 1) * hd].astype(f8)
                      @ wk_c[h].astype(f8)).astype(np.float32)
    wv_h = lambda h: w_attn[:, 2 * D + h * hd:2 * D + (h + 1) * hd]
    bv_h = lambda h: b_attn[2 * D + h * hd:2 * D + (h + 1) * hd]
    wpj_h = lambda h: (wv_d[h].astype(f8)
                       @ w_proj[h * hd:(h + 1) * hd, :].astype(f8)).astype(np.float32)

    k = np.arange(128).reshape(128, 1)
    j = np.arange(128).reshape(1, 128)
    mask = (k <= j).astype(BF16)

    in_maps = []
    for c in range(NCORES):
        b = c // 4
        hs = list(range((c % 4) * HPC, (c % 4) * HPC + HPC))
        # m-blocks: [q'0..3], [kc0..3], [v0|v1], [v2|v3]
        cols = ([wq_h(h) for h in hs] + [wk_h(h) for h in hs]
                + [wv_h(h) for h in hs])
        w_qkv_l = np.concatenate(cols, axis=1).astype(BF16)        # [1024, 512]
        bcols = ([bq_h(h) for h in hs] + [bk_h(h) for h in hs]
                 + [bv_h(h) for h in hs])
        b_qkv_l = (np.concatenate(bcols).astype(np.float32)
                   .reshape(4, 128).T.copy())                      # [128, 4]
        # block-diag wv_c per pair: rows 0-63 even head, 64-127 odd head
        wvc2_l = np.zeros((128, 128), np.float32)
        for p in range(2):
            wvc2_l[0:hd, p * 64:p * 64 + C] = wv_c[hs[2 * p]]
            wvc2_l[hd:128, p * 64 + C:p * 64 + 2 * C] = wv_c[hs[2 * p + 1]]
        # w_proj' pairs: rows 0-31 even head, 64-95 odd head, rest zero
        wpj_l = np.zeros((2, 128, D), np.float32)
        for p in range(2):
            wpj_l[p, 0:C, :] = wpj_h(hs[2 * p])
            wpj_l[p, 64:64 + C, :] = wpj_h(hs[2 * p + 1])
        in_maps.append(
            {
                "hidden_t": hidden_T[b],
                "w_qkv": w_qkv_l,
                "b_qkv": b_qkv_l,
                "wvc2": wvc2_l.astype(BF16),
                "wpj": wpj_l.astype(BF16),
                "maskd": np.ascontiguousarray(mask),
            }
        )
    return in_maps


def kernel(
    hidden_states,
    w_attn,
    b_attn,
    w_proj,
    b_proj,
    wk_c,
    wv_c,
    wk_d,
    wv_d,
    _trace=False,
):
    from concourse.bass_utils import run_bass_kernel_spmd

    if "nc" not in _cache:
        _cache["nc"] = _build()
    nc = _cache["nc"]

    in_maps = _prep_inputs(
        np.asarray(hidden_states),
        np.asarray(w_attn),
        np.asarray(b_attn),
        np.asarray(wk_c),
        np.asarray(wv_c),
        np.asarray(wk_d),
        np.asarray(wv_d),
        np.asarray(w_proj),
    )
    res = run_bass_kernel_spmd(
        nc, in_maps, core_ids=list(range(NCORES)), trace=_trace
    )
    out = np.empty((B, S, D), np.float32)
    for b in range(B):
        acc = np.zeros((D, S), np.float32)
        for c in range(4 * b, 4 * b + 4):
            acc += res.results[c]["out_t"].astype(np.float32)
        out[b] = acc.T + np.asarray(b_proj, np.float32)
    if _trace:
        _cache["last_exec_time_ns"] = res.exec_time_ns
        _cache["last_results"] = res
    return out

